# revision 2
# baseline (speedup 1.0000x reference)
"""BsPINN forward MLP on 8 Trainium2 NeuronCores (data-parallel over batch).

Network (reference): X[65536,2] -> normalize -> tanh(XW0+b0) -> tanh(hW1+b1)
  -> tanh(h(W2*mask2)+b2) -> tanh(h(W3*mask3)+b3) -> hW_last+b_last -> [65536,1]
mask2 = blockdiag(2 x [512x512] ones), mask3 = blockdiag(4 x [256x256] ones).

Device strategy (per core, 8192 rows, row-tiles of 1024):
  - Activations kept TRANSPOSED in SBUF: [features(partitions), rows(free)].
    Dense layers are psum[m] += W[kblk,mblk].T @ hT[kblk] on the PE
    (out = lhsT.T @ rhs), all matmul operands in FP16: same 1 cycle/row as
    f32r but half the weight-load (LDWEIGHTS) time and half the xbus
    pressure, which removes the ~23ns/matmul un-overlapped weight-load
    residual seen in the f32r trace (end-to-end quantization rel-err ~2.5e-3
    vs the 2e-2 gate, measured in numpy sim).
  - Input normalization folded into W0/b0 on the host; layer 0 (K=2) runs on
    the idle Vector engine (x broadcast via 0-stride DMA) instead of wasting
    the 128x128 PE array on K=2 matmuls.
  - Masked layers packed on host to only their nonzero 128-blocks
    (layer2: 4 k-blocks per m-tile, layer3: 2).
  - bias+tanh fused into one ScalarE activation per [128,1024] psum pair.
  - Output layer moved OFF the PE: t4[p,r] = sum_k wl[p,k]*h3_k[p,r] built
    with 8 DVE fma ops (interleaved into P1 right after each h3 m-group), then
    a single ones-vector partition-reduce matmul per 512-row half (2x512 PE
    cycles/row-tile instead of 16x512).
  - Two-phase software pipeline across row-tiles so matmul-dense work always
    overlaps the tanh-heavy psum drains (keeps PE busy and its clock warm):
      P1(r): L1(r) m0..6 interleaved with L3(r-1) + L4 fma chain (DVE)
      P2(r): L1(r) m7, L2(r), L4-reduce(r-1), and layer-0(r+1) on DVE
"""

import os

import numpy as np

N_CORES = 8
N_ROWS = 65536
R = N_ROWS // N_CORES  # rows per core
ROWS_T = 1024  # rows per row-tile (psum pair = 2 banks per m-tile)
N_RT = R // ROWS_T
H = 1024
P = 128
KT = H // P  # 8 feature tiles

_STATE = {}


def _build_module():
    import concourse.bacc as bacc
    import concourse.mybir as mybir
    import concourse.tile as tile

    f32 = mybir.dt.float32
    f32r = mybir.dt.float32r
    f16 = mybir.dt.float16
    Tanh = mybir.ActivationFunctionType.Tanh
    Mult = mybir.AluOpType.mult
    Add = mybir.AluOpType.add

    nc = bacc.Bacc("TRN2", target_bir_lowering=False, debug=False)

    xT = nc.dram_tensor("xT", [2, R], f16, kind="ExternalInput")
    w0 = nc.dram_tensor("w0", [P, 2 * KT], f32, kind="ExternalInput")
    w1 = nc.dram_tensor("w1", [H, H], f16, kind="ExternalInput")
    w2 = nc.dram_tensor("w2", [H, 512], f16, kind="ExternalInput")
    w3 = nc.dram_tensor("w3", [H, 256], f16, kind="ExternalInput")
    wl = nc.dram_tensor("wl", [P, KT], f32, kind="ExternalInput")
    bt = nc.dram_tensor("bt", [P, 4 * KT], f32, kind="ExternalInput")
    x0r = nc.dram_tensor("x0r", [2, ROWS_T], f32r, kind="ExternalInput")
    onesd = nc.dram_tensor("onesd", [P, 1], f32r, kind="ExternalInput")
    w0k = nc.dram_tensor("w0k", [2, H], f32r, kind="ExternalInput")
    outT = nc.dram_tensor("outT", [1, R], f32, kind="ExternalOutput")

    with tile.TileContext(nc) as tc:
        with (
            tc.tile_pool(name="wpool", bufs=1) as wp,
            tc.tile_pool(name="hpool", bufs=1) as hp,
            tc.tile_pool(name="xpool", bufs=1) as xp,
            tc.tile_pool(name="opool", bufs=1) as op,
            tc.tile_pool(name="psum", bufs=4, space="PSUM") as pp,
        ):
            bts = wp.tile([P, 4 * KT], f32, tag="bt")
            w0s = wp.tile([P, 2 * KT], f32, tag="w0")
            ones = wp.tile([P, 1], f32r, tag="ones")
            nc.sync.dma_start(ones[:], onesd[:])

            # PE warmup: the HAM clock gate holds the PE at 1.2 GHz until it
            # has been busy ~3.4us. Issue dummy matmuls at t=0 (while input
            # DMAs are in flight) so the real prologue runs at 2.4 GHz.
            # Also fire a dummy activation so the Tanh table load (~1.3us)
            # is off the first real tanh's critical path.
            wu_w = wp.tile([P, P], f16, tag="wuw")
            wu_h = wp.tile([P, 512], f16, tag="wuh")
            nc.vector.memset(wu_w[:], 0.0)
            nc.vector.memset(wu_h[:], 0.0)
            wu_act = hp.tile([P, 1], f16, tag="wua")
            nc.scalar.activation(wu_act[:], wu_w[:, 0:1], Tanh)
            wu_ps = pp.tile([P, 512], f32, tag="ps", name="wups")
            for _ in range(10):
                nc.tensor.matmul(wu_ps[:], wu_w[:], wu_h[:], start=True, stop=True)

            xbs = [None] * (N_RT + 1)

            def load_xb(r):
                if r < N_RT:
                    rs = r * ROWS_T
                    b0 = xp.tile([P, ROWS_T], f16, tag="xb0", name=f"xb0_{r}")
                    b1 = xp.tile([P, ROWS_T], f16, tag="xb1", name=f"xb1_{r}")
                    nc.gpsimd.dma_start(b0[:], xT[0, rs : rs + ROWS_T].partition_broadcast(P))
                    nc.gpsimd.dma_start(b1[:], xT[1, rs : rs + ROWS_T].partition_broadcast(P))
                    xbs[r] = (b0, b1)

            xs0t = hp.tile([P, ROWS_T], f32, tag="boot1", name="xs0boot")
            w0kt = hp.tile([P, ROWS_T], f32, tag="boot2", name="w0kboot")
            nc.sync.dma_start(xs0t[0:2, :].bitcast(f32r), x0r[:])
            nc.scalar.dma_start(w0kt[0:2, :].bitcast(f32r), w0k[:])
            nc.sync.dma_start(bts[:], bt[:])
            nc.sync.dma_start(w0s[:], w0[:])
            w1s = []
            w2s = []
            w3s = []
            w1engs = [nc.sync, nc.gpsimd, nc.scalar]
            for k in range(KT):
                t = wp.tile([P, H], f16, tag=f"w1_{k}")
                w1engs[k % 3].dma_start(t[:], w1[k * P : (k + 1) * P, :])
                w1s.append(t)
            for k in range(KT):
                t = wp.tile([P, 512], f16, tag=f"w2_{k}")
                eng = nc.sync if k % 2 == 0 else nc.gpsimd
                eng.dma_start(t[:], w2[k * P : (k + 1) * P, :])
                w2s.append(t)
            load_xb(0)
            for k in range(KT):
                t = wp.tile([P, 256], f16, tag=f"w3_{k}")
                nc.sync.dma_start(t[:], w3[k * P : (k + 1) * P, :])
                w3s.append(t)
            wls = wp.tile([P, KT], f32, tag="wl")
            nc.sync.dma_start(wls[:], wl[:])

            hs = {}  # (layer, r) -> list of 8 tiles

            def dense_group(layer, m, wk, hin, hout_tag, ks, mo, split_act=False):
                """One m-tile group: PE matmuls over ks into a psum pair + tanh.

                split_act=True drains each 512-half with its own activation as
                soon as that half's matmuls are done (shorter critical path for
                the epilogue row-tile, at +352 ScalarE overhead cycles/half).
                """
                ps = pp.tile([P, ROWS_T], f32, tag="ps", name=f"ps{layer}_{m}")
                ht = hp.tile([P, ROWS_T], f16, tag=f"{hout_tag}_{m}", name=f"{hout_tag}_{m}")
                bias = bts[:, layer * KT + m : layer * KT + m + 1]
                for hf in range(2):
                    c = hf * 512
                    for j, k in enumerate(ks):
                        nc.tensor.matmul(
                            ps[:, c : c + 512],
                            wk[k][:, mo(k) : mo(k) + P] if callable(mo) else wk[k][:, mo : mo + P],
                            hin[k][:, c : c + 512],
                            start=(j == 0),
                            stop=(j == len(ks) - 1),
                        )
                    if split_act:
                        nc.scalar.activation(
                            ht[:, c : c + 512], ps[:, c : c + 512], Tanh, bias=bias
                        )
                if not split_act:
                    nc.scalar.activation(ht[:], ps[:], Tanh, bias=bias)
                return ht

            def l0_group(r, m):
                """Layer 0 on DVE: h0[m] = tanh(x0*w0c0[m] + x1*w0c1[m] + b0[m])."""
                b0, b1 = xbs[r]
                t1 = hp.tile([P, ROWS_T], f16, tag="t1", name=f"t1_{m}")
                t2 = hp.tile([P, ROWS_T], f16, tag="t2", name=f"t2_{m}")
                nc.vector.tensor_scalar_mul(t1[:], b0[:], w0s[:, m : m + 1])
                nc.vector.scalar_tensor_tensor(
                    t2[:], b1[:], w0s[:, KT + m : KT + m + 1], t1[:], Mult, Add
                )
                ht = hp.tile([P, ROWS_T], f16, tag=f"h0_{m}", name=f"h0_{m}")
                nc.scalar.activation(ht[:], t2[:], Tanh, bias=bts[:, m : m + 1])
                return ht

            t4s = {}

            def l4_fma(r, k, hf=None):
                """Accumulate t4 += h3_k * wl_k on DVE (one fma per k).

                hf selects a 512-half (epilogue: lets the half-reduce start
                before the other half's chain completes)."""
                sl = slice(0, ROWS_T) if hf is None else slice(hf * 512, hf * 512 + 512)
                if k == 0:
                    if hf is None or hf == 0:
                        t4s[r] = op.tile([P, ROWS_T], f32r, tag="t4", name=f"t4_{r}")
                    nc.vector.tensor_scalar_mul(
                        t4s[r][:, sl], hs[(3, r)][0][:, sl], wls[:, 0:1]
                    )
                else:
                    nc.vector.scalar_tensor_tensor(
                        t4s[r][:, sl], hs[(3, r)][k][:, sl], wls[:, k : k + 1],
                        t4s[r][:, sl], Mult, Add,
                    )

            def l4_reduce(r, hf, ot):
                c = hf * 512
                psl = pp.tile([1, 512], f32, tag="ps", name=f"psl{hf}")
                nc.tensor.matmul(
                    psl[:],
                    ones[:],
                    t4s[r][:, c : c + 512],
                    start=True,
                    stop=True,
                )
                nc.vector.tensor_copy(ot[0:1, c : c + 512], psl[:])

            # prologue: layer 0 of row-tile 0 on the PE (fast startup; the
            # steady-state layer 0 runs on DVE via l0_group)
            def l0_pe_group(m):
                ps = pp.tile([P, ROWS_T], f32, tag="ps", name=f"psb_{m}")
                for hf in range(2):
                    c = hf * 512
                    nc.tensor.matmul(
                        ps[:, c : c + 512],
                        w0kt[0:2, m * P : (m + 1) * P].bitcast(f32r),
                        xs0t[0:2, c : c + 512].bitcast(f32r),
                        start=True,
                        stop=True,
                    )
                ht = hp.tile([P, ROWS_T], f16, tag=f"h0_{m}", name=f"h0_{m}")
                nc.scalar.activation(ht[:], ps[:], Tanh, bias=bts[:, m : m + 1])
                return ht

            hs[(0, 0)] = [l0_pe_group(m) for m in range(KT)]

            ots = {}
            for r in range(N_RT + 1):
                # ---- phase P1(r): L1(r) m0..6 interleaved with L3(r-1) ----
                if r == N_RT:
                    # epilogue: the final row-tile's L3+L4 were fused into
                    # P2(N_RT-1); only its reduce + output DMA remain.
                    ots[r - 1] = op.tile([P, ROWS_T], f32, tag="o", name=f"ot{r-1}")
                    l4_reduce(r - 1, 0, ots[r - 1])
                    l4_reduce(r - 1, 1, ots[r - 1])
                    rs = (r - 1) * ROWS_T
                    nc.sync.dma_start(outT[:, rs : rs + ROWS_T], ots[r - 1][0:1, :])
                    break
                load_xb(r + 1)
                hs[(1, r)] = [None] * KT
                if r >= 1:
                    hs[(3, r - 1)] = [None] * KT
                for m in range(KT):
                    if m < KT - 1:
                        hs[(1, r)][m] = dense_group(
                            1, m, w1s, hs[(0, r)], "h1", list(range(KT)), m * P
                        )
                    if r >= 1:
                        hs[(3, r - 1)][m] = dense_group(
                            3, m, w3s, hs[(2, r - 1)],
                            "h3", [(m // 2) * 2 + j for j in range(2)], ((m % 2) * P),
                        )
                        l4_fma(r - 1, m)
                if r >= 1:
                    hs.pop((2, r - 1), None)

                # ---- phase P2(r): L1(r) m7, L2(r), L4-reduce(r-1), L0(r+1);
                # for the last row-tile, L3(r)+L4-fma(r) are interleaved here
                # too (offset -2 behind L2) so no separate drain phase remains.
                fuse = r == N_RT - 1
                hs[(2, r)] = [None] * KT
                hs[(1, r)][KT - 1] = dense_group(
                    1, KT - 1, w1s, hs[(0, r)], "h1", list(range(KT)), (KT - 1) * P
                )
                if r >= 1:
                    hs.pop((0, r - 1), None)
                if r + 1 < N_RT:
                    hs[(0, r + 1)] = [None] * KT
                if r >= 1:
                    ots[r - 1] = op.tile([P, ROWS_T], f32, tag="o", name=f"ot{r-1}")
                if fuse:
                    hs[(3, r)] = [None] * KT

                def l3_fused(m3):
                    hs[(3, r)][m3] = dense_group(
                        3, m3, w3s, hs[(2, r)],
                        "h3", [(m3 // 2) * 2 + j for j in range(2)], ((m3 % 2) * P),
                        split_act=(m3 >= KT - 2),
                    )
                    if m3 >= KT - 2:
                        l4_fma(r, m3, 0)
                        l4_fma(r, m3, 1)
                    else:
                        l4_fma(r, m3)

                for m in range(KT):
                    hs[(2, r)][m] = dense_group(
                        2, m, w2s, hs[(1, r)],
                        "h2", [(m // 4) * 4 + j for j in range(4)], ((m % 4) * P),
                    )
                    if r + 1 < N_RT:
                        hs[(0, r + 1)][m] = l0_group(r + 1, m)
                    if fuse and m >= 2:
                        l3_fused(m - 2)
                    if r >= 1 and m == 1:
                        l4_reduce(r - 1, 0, ots[r - 1])
                    if r >= 1 and m == 4:
                        l4_reduce(r - 1, 1, ots[r - 1])
                if fuse:
                    l3_fused(KT - 2)
                    l3_fused(KT - 1)
                if r >= 1:
                    rs = (r - 1) * ROWS_T
                    nc.sync.dma_start(outT[:, rs : rs + ROWS_T], ots[r - 1][0:1, :])
                    hs.pop((3, r - 1), None)
                    t4s.pop(r - 1, None)
                if r >= 1:
                    hs.pop((1, r - 1), None)

    nc.compile()
    return nc


def _get_module():
    if "nc" not in _STATE:
        _STATE["nc"] = _build_module()
    return _STATE["nc"]


def _tcol(v, dt=np.float32):
    """[1024] vector -> [128, 8]: column t holds v[t*128:(t+1)*128]."""
    return np.ascontiguousarray(v.reshape(KT, P).T).astype(dt)


def prep_in_maps(inputs):
    X = np.asarray(inputs["X"], np.float32)
    W0 = np.asarray(inputs["W0"], np.float32)
    b0 = np.asarray(inputs["b0"], np.float32)
    W1 = np.asarray(inputs["W1"], np.float32)
    b1 = np.asarray(inputs["b1"], np.float32)
    W2 = np.asarray(inputs["W2"], np.float32)
    b2 = np.asarray(inputs["b2"], np.float32)
    W3 = np.asarray(inputs["W3"], np.float32)
    b3 = np.asarray(inputs["b3"], np.float32)
    Wl = np.asarray(inputs["W_last"], np.float32)

    # fold `h = 2*(X-LB)/(UB-LB) - 1` (LB=[0,0], UB=[2pi,1]) into layer 0
    s = np.array([1.0 / np.pi, 2.0], np.float32)
    w0p = s[:, None] * W0
    b0p = b0[0] - W0[0] - W0[1]
    # layer-0 weights in per-partition layout for DVE: [128, 16]
    w0t = np.ascontiguousarray(
        np.concatenate([_tcol(w0p[0]), _tcol(w0p[1])], axis=1)
    )

    # pack only the nonzero 128-blocks of the masked layers
    w2p = np.ascontiguousarray(
        np.concatenate(
            [W2[k * P : (k + 1) * P, (k // 4) * 512 : (k // 4) * 512 + 512] for k in range(KT)],
            axis=0,
        )
    ).astype(np.float16)
    w3p = np.ascontiguousarray(
        np.concatenate(
            [W3[k * P : (k + 1) * P, (k // 2) * 256 : (k // 2) * 256 + 256] for k in range(KT)],
            axis=0,
        )
    ).astype(np.float16)
    wlp = _tcol(Wl[:, 0])
    btp = np.ascontiguousarray(
        np.concatenate([_tcol(b0p), _tcol(b1[0]), _tcol(b2[0]), _tcol(b3[0])], axis=1)
    )

    xT = np.ascontiguousarray(X.T)  # [2, 65536]
    xT16 = xT.astype(np.float16)
    common = {
        "w0": w0t, "w1": W1.astype(np.float16), "w2": w2p, "w3": w3p,
        "wl": wlp, "bt": btp,
        "w0k": np.ascontiguousarray(w0p),
        "onesd": np.ones((P, 1), np.float32),
    }
    return [
        {
            "xT": np.ascontiguousarray(xT16[:, c * R : (c + 1) * R]),
            "x0r": np.ascontiguousarray(xT[:, c * R : c * R + ROWS_T]),
            **common,
        }
        for c in range(N_CORES)
    ]


def kernel(**inputs):
    from concourse.bass_utils import run_bass_kernel_spmd

    nc = _get_module()
    in_maps = prep_in_maps(inputs)
    trace = bool(int(os.environ.get("BASS_KERNEL_TRACE", "0")))
    res = run_bass_kernel_spmd(nc, in_maps, list(range(N_CORES)), trace=trace)
    _STATE["last_result"] = res
    out = np.concatenate([res.results[c]["outT"].reshape(-1) for c in range(N_CORES)])
    b_last = np.asarray(inputs["b_last"], np.float32)
    return (out.reshape(-1, 1) + b_last).astype(np.float32)
